# revision 9
# baseline (speedup 1.0000x reference)
"""MixConv depthwise conv (3x3/5x5/7x7 over 64-channel groups) as banded-Toeplitz
matmuls on the TensorEngine, sharded over 8 NeuronCores by channel.

Decomposition: a kxk depthwise conv = sum over dx of a 1D conv along H applied to
the input shifted by dx along W. The 1D conv along H is a matmul with a banded
[H, H] Toeplitz matrix (built host-side from the conv weights) contracting over
H=112 partitions. W-shifts are free-dim offsets into a padded SBUF image tile;
the dx-passes accumulate in PSUM.

Sharding: 192 channels / 8 cores = 24 channels per core, 8 from each kernel-size
group so PE work is balanced. Weights (Toeplitz form, ~6MB/core) ride along as an
extra input. Host stages x into the padded per-channel layout so every device DMA
is a dense 2D copy.

Matmuls run in fp32r (1 cycle/row vs fp32's 4): hardware RNE-rounds both
operands to 11-bit mantissa and accumulates exactly in fp32 PSUM — measured
~1.5e-4 scale-relative output error.
"""

import numpy as np

import concourse.bacc as bacc
import concourse.mybir as mybir
import concourse.tile as tile
from concourse.bass_utils import run_bass_kernel_spmd

# Problem constants (hardcoded per contract)
N_IMGS = 32
H = W = 112
GROUP_KS = (3, 5, 7)
GROUP_SIZE = 64          # channels per group
N_CORES = 8
CH_PER_GROUP_PER_CORE = GROUP_SIZE // N_CORES   # 8
CH_PER_CORE = CH_PER_GROUP_PER_CORE * len(GROUP_KS)  # 24

RW = W + 6               # per-image region width in the padded tile (max pad=3)
DATA_OFF = 3             # data cols at [3, 115) of each region
XCOLS = N_IMGS * RW + 8  # +8 slack for last-chunk matmul over-read
OCOLS = N_IMGS * W
N_MM = 4 * RW            # 472 — matmul free dim (4 images/chunk), even (fp32r)

KS = [3] * 8 + [5] * 8 + [7] * 8          # per-channel kernel size (core order)
TOFF = np.cumsum([0] + KS).tolist()       # tmat row offset per channel
N_TMAT = TOFF[-1]                          # 120

# "fp32": exact, 4 cyc/row.  "fp32r": 1 cyc/row, ~1.5e-4 rel err.
# "fp32r_split": weights split hi/lo, 2 fp32r passes — ~0.7e-4, 2 cyc/row.
MM_MODE = "fp32r"

_BASS_CACHE = {}


def _build_bass(mode):
    nsplit = 2 if mode == "fp32r_split" else 1
    use_f32r = mode in ("fp32r", "fp32r_split")
    mm_dt = mybir.dt.float32r if use_f32r else mybir.dt.float32
    f32 = mybir.dt.float32

    nc = bacc.Bacc("TRN2", target_bir_lowering=False, debug=False)
    xp_d = nc.dram_tensor("xp", [CH_PER_CORE, H, XCOLS], f32, kind="ExternalInput")
    t_d = nc.dram_tensor("tmat", [nsplit * N_TMAT, H, H], f32, kind="ExternalInput")
    y_d = nc.dram_tensor("y", [CH_PER_CORE, H, OCOLS], f32, kind="ExternalOutput")

    def src(ap):
        return ap.bitcast(mm_dt) if use_f32r else ap

    with tile.TileContext(nc) as tc:
        with (
            tc.tile_pool(name="xpool", bufs=2) as xpool,
            tc.tile_pool(name="tpool", bufs=2) as tpool,
            tc.tile_pool(name="opool", bufs=2) as opool,
            tc.tile_pool(name="pspool", bufs=8, space="PSUM") as pspool,
        ):
            for ch in range(CH_PER_CORE):
                k = KS[ch]
                pad = (k - 1) // 2
                x_t = xpool.tile([H, XCOLS], mm_dt, tag="x", name=f"x{ch}")
                nc.sync.dma_start(x_t[:, :], src(xp_d[ch]))
                t_t = tpool.tile([H, nsplit * 7 * H], mm_dt, tag="t", name=f"t{ch}")
                for s in range(nsplit):
                    nc.sync.dma_start(
                        t_t[:, s * k * H : (s + 1) * k * H].rearrange(
                            "p (d m) -> p d m", d=k
                        ),
                        src(
                            t_d[
                                s * N_TMAT + TOFF[ch] : s * N_TMAT + TOFF[ch] + k
                            ].rearrange("d hin hout -> hin d hout")
                        ),
                    )
                out_t = opool.tile([H, OCOLS], f32, tag="o", name=f"o{ch}")
                passes = [(s, dx) for s in range(nsplit) for dx in range(k)]
                for half in range(2):
                    pts = [
                        pspool.tile(
                            [H, N_MM], f32, tag="ps", name=f"ps{ch}_{half}_{b}"
                        )
                        for b in range(4)
                    ]
                    for pi, (s, dx) in enumerate(passes):
                        off = dx - pad + DATA_OFF
                        lhsT = t_t[:, (s * k + dx) * H : (s * k + dx + 1) * H]
                        for b in range(4):
                            base = (16 * half + 4 * b) * RW
                            nc.tensor.matmul(
                                pts[b],
                                lhsT=lhsT,
                                rhs=x_t[:, base + off : base + off + N_MM],
                                start=(pi == 0),
                                stop=(pi == len(passes) - 1),
                            )
                    for b in range(4):
                        img0 = 16 * half + 4 * b
                        nc.any.tensor_copy(
                            out=out_t.rearrange("p (i w) -> p i w", i=N_IMGS)[
                                :, img0 : img0 + 4, :
                            ],
                            in_=pts[b].rearrange("p (i r) -> p i r", i=4)[:, :, :W],
                        )
                nc.sync.dma_start(y_d[ch], out_t[:, :])
    nc.compile()
    return nc


def _get_bass(mode):
    if mode not in _BASS_CACHE:
        _BASS_CACHE[mode] = _build_bass(mode)
    return _BASS_CACHE[mode]


def _build_toeplitz(w, k):
    """w: [C, 1, k, k] -> T: [C, k, H, H], T[c,dx,hin,hout] = w[c,0,hin-hout+pad,dx]."""
    pad = (k - 1) // 2
    C = w.shape[0]
    T = np.zeros((C, k, H, H), np.float32)
    for dy in range(k):
        off = pad - dy  # hout = hin + off
        hin = np.arange(max(0, -off), H - max(0, off))
        T[:, :, hin, hin + off] = w[:, 0, dy, :][:, :, None]
    return T


def _round_fp32r(a):
    """RNE round fp32 to 11-bit mantissa (the fp32r grid) — matches HW."""
    u = a.astype(np.float32).view(np.uint32).astype(np.uint64)
    lsb = (u >> 12) & 1
    u = (u + 0x7FF + lsb) & 0xFFFFF000
    return u.astype(np.uint32).view(np.float32)


def _core_channels(core):
    out = []
    for g in range(len(GROUP_KS)):
        base = g * GROUP_SIZE + core * CH_PER_GROUP_PER_CORE
        out.extend(range(base, base + CH_PER_GROUP_PER_CORE))
    return out


def _prepare_in_maps(x, w3, w5, w7, mode):
    x = np.ascontiguousarray(np.asarray(x, dtype=np.float32))
    ws = {3: np.asarray(w3, np.float32), 5: np.asarray(w5, np.float32),
          7: np.asarray(w7, np.float32)}
    Ts = {k: _build_toeplitz(ws[k], k) for k in GROUP_KS}

    in_maps = []
    for core in range(N_CORES):
        chs = _core_channels(core)
        # staged x: [24, H, XCOLS], data at [i*RW+3, i*RW+115) per image
        xp = np.zeros((CH_PER_CORE, H, XCOLS), np.float32)
        xv = xp[:, :, : N_IMGS * RW].reshape(CH_PER_CORE, H, N_IMGS, RW)
        xv[:, :, :, DATA_OFF : DATA_OFF + W] = x[:, chs].transpose(1, 2, 0, 3)

        tm = np.concatenate(
            [
                Ts[GROUP_KS[g]][
                    core * CH_PER_GROUP_PER_CORE : (core + 1) * CH_PER_GROUP_PER_CORE
                ].reshape(-1, H, H)
                for g in range(len(GROUP_KS))
            ],
            axis=0,
        )
        assert tm.shape[0] == N_TMAT
        if mode == "fp32r_split":
            hi = _round_fp32r(tm)
            lo = tm - hi
            tm = np.concatenate([hi, lo], axis=0)
        in_maps.append({"xp": xp, "tmat": np.ascontiguousarray(tm)})
    return in_maps


def _gather(results):
    out = np.empty((N_IMGS, GROUP_SIZE * len(GROUP_KS), H, W), np.float32)
    for core in range(N_CORES):
        chs = _core_channels(core)
        y = results[core]["y"].reshape(CH_PER_CORE, H, N_IMGS, W)
        out[:, chs] = y.transpose(2, 0, 1, 3)
    return out


def run(x, w3, w5, w7, **spmd_kwargs):
    """Full run; returns (output, BassKernelResults) for profiling access."""
    nc = _get_bass(MM_MODE)
    in_maps = _prepare_in_maps(x, w3, w5, w7, MM_MODE)
    br = run_bass_kernel_spmd(nc, in_maps, core_ids=list(range(N_CORES)), **spmd_kwargs)
    return _gather(br.results), br


def kernel(x, w3, w5, w7):
    out, _ = run(x, w3, w5, w7)
    return out
